# revision 6
# baseline (speedup 1.0000x reference)
"""Trainium2 Bass kernel for nn_LinearBlock (block-structured linear + ReLUN).

y = clip(x @ W^T, 0, 6) where W is assembled from [128,128,8,8] blocks.

Strategy: data-parallel over the 16384-token batch dim, 2048 tokens/core on
8 cores, weight replicated. Per core:
  - W^T ([in=1024, out=1024], host-assembled) is DMA'd once, rounded to
    fp32r in SBUF (fp32r matmuls run at full PE rate; plain fp32 is 4x
    slower).
  - For each 128-token tile: DMA the x tile [128, 1024], PE-transpose its
    eight [128,128] sub-tiles (contraction dim must be on partitions),
    evict pairs from PSUM with DVE copies (which round to fp32r), then 16
    fp32r matmuls (free dim 512) accumulate over the 8 contraction tiles
    into PSUM [128 tok, 1024 out].
  - One DVE tensor_scalar (max 0, min 6) evicts PSUM -> SBUF with the
    ReLUN fused, then DMA out.
  - DMA traffic alternates between the two TRN2 HWDGE queues (sync,
    scalar) — a single queue's descriptor dispatch caps at ~170 GB/s.
"""

import numpy as np

N_TOKENS = 16384
D_IN = 1024
D_OUT = 1024
N_CORES = 8
TOK = N_TOKENS // N_CORES  # 2048 tokens per core
P = 128
K_TILES = D_IN // P   # 8
T_TILES = TOK // P    # 16
FD = 512              # matmul free-dim chunk (one PSUM bank of fp32)
PIPE = 4              # token tiles of transpose lookahead ahead of matmul

_CACHE = {}
LAST_RESULTS = None


def _build():
    import concourse.bacc as bacc
    import concourse.tile as tile
    import concourse.mybir as mybir
    from concourse.masks import make_identity

    f32 = mybir.dt.float32
    f32r = mybir.dt.float32r

    nc = bacc.Bacc("TRN2", target_bir_lowering=False, debug=False)
    x = nc.dram_tensor("x", [TOK, D_IN], f32, kind="ExternalInput").ap()
    wt = nc.dram_tensor("wt", [D_IN, D_OUT], f32, kind="ExternalInput").ap()
    y = nc.dram_tensor("y", [TOK, D_OUT], f32, kind="ExternalOutput").ap()

    def dma_eng(parity):
        return nc.sync if parity == 0 else nc.scalar

    with tile.TileContext(nc) as tc:
        with (
            tc.tile_pool(name="cpool", bufs=1) as cpool,
            tc.tile_pool(name="wpool", bufs=1) as wpool,
            tc.tile_pool(name="xpool", bufs=4) as xpool,
            tc.tile_pool(name="xtpool", bufs=10) as xtpool,
            tc.tile_pool(name="ypool", bufs=3) as ypool,
            tc.tile_pool(name="pstpool", bufs=4, space="PSUM") as pstpool,
            tc.tile_pool(name="psypool", bufs=2, space="PSUM") as psypool,
        ):
            ident = cpool.tile([P, P], f32)
            make_identity(nc, ident)

            # Full replicated weight in SBUF: [128 (i within tile), k-tile, out]
            w_sb = wpool.tile([P, K_TILES, D_OUT], f32r)

            xt_of = {}    # t -> list of 4 [128, 256] fp32r tiles (k-pairs)
            psy_of = {}   # t -> accumulation PSUM tile

            def emit_load_and_transpose(t):
                x_sb = xpool.tile([P, D_IN], f32, tag="x")
                dma_eng(t % 2).dma_start(x_sb[:], x[t * P:(t + 1) * P, :])
                if t == 0:
                    # Weight loads right after the first x tile on each
                    # queue; matmuls consume w_k in k order. Casts to fp32r
                    # run on GpSimd (SBUF->SBUF, 1-input is line-rate) to
                    # keep DVE free for PSUM evictions.
                    for k in range(K_TILES):
                        w_raw = xpool.tile([P, D_OUT], f32, tag="wraw", bufs=4)
                        dma_eng(k % 2).dma_start(
                            w_raw[:], wt[k * P:(k + 1) * P, :])
                        nc.gpsimd.tensor_copy(out=w_sb[:, k], in_=w_raw[:])
                pairs = []
                for kp in range(K_TILES // 4):
                    pst = pstpool.tile([P, 4 * P], f32, tag="pst")
                    for j in range(4):
                        k = 4 * kp + j
                        nc.tensor.transpose(
                            pst[:, j * P:(j + 1) * P],
                            x_sb[:, k * P:(k + 1) * P], ident[:])
                    xt = xtpool.tile([P, 4 * P], f32r, tag="xt")
                    nc.vector.tensor_copy(out=xt[:], in_=pst[:])
                    pairs.append(xt)
                xt_of[t] = pairs

            def emit_matmuls(t):
                psy = psypool.tile([P, D_OUT], f32, tag="psy")
                for k in range(K_TILES):
                    lhsT = xt_of[t][k // 4][:, (k % 4) * P:(k % 4 + 1) * P]
                    for c in range(D_OUT // FD):
                        nc.tensor.matmul(
                            psy[:, c * FD:(c + 1) * FD],
                            lhsT,
                            w_sb[:, k, c * FD:(c + 1) * FD],
                            start=(k == 0),
                            stop=(k == K_TILES - 1),
                        )
                del xt_of[t]
                psy_of[t] = psy

            def emit_store(t):
                psy = psy_of.pop(t)
                y_sb = ypool.tile([P, D_OUT], f32, tag="y")
                nc.vector.tensor_scalar(
                    y_sb[:], psy[:], 0.0, 6.0,
                    mybir.AluOpType.max, mybir.AluOpType.min,
                )
                dma_eng((t + 1) % 2).dma_start(y[t * P:(t + 1) * P, :], y_sb[:])

            for t in range(T_TILES):
                emit_load_and_transpose(t)
                if t >= PIPE:
                    emit_matmuls(t - PIPE)
                    emit_store(t - PIPE)
            for t in range(T_TILES - PIPE, T_TILES):
                emit_matmuls(t)
                emit_store(t)

    nc.compile()
    return nc


def kernel(x, w_blocks, _trace=False):
    global LAST_RESULTS
    from concourse import bass_utils

    nc = _CACHE.get("nc")
    if nc is None:
        nc = _CACHE["nc"] = _build()

    x = np.ascontiguousarray(np.asarray(x, dtype=np.float32))
    wb = np.asarray(w_blocks, dtype=np.float32)
    # W[8a+r, 8b+s] = wb[a, b, r, s]; wt[i, o] = W[o, i]
    wt = np.ascontiguousarray(wb.transpose(1, 3, 0, 2).reshape(D_IN, D_OUT))

    in_maps = [
        {"x": x[c * TOK:(c + 1) * TOK], "wt": wt} for c in range(N_CORES)
    ]
    res = bass_utils.run_bass_kernel_spmd(
        nc, in_maps, core_ids=list(range(N_CORES)), trace=_trace,
    )
    LAST_RESULTS = res
    return np.concatenate([res.results[c]["y"] for c in range(N_CORES)], axis=0)


# revision 7
# speedup vs baseline: 1.1607x; 1.1607x over previous
"""Trainium2 Bass kernel for nn_LinearBlock (block-structured linear + ReLUN).

y = clip(x @ W^T, 0, 6) where W is assembled from [128,128,8,8] blocks.

Strategy: data-parallel over the 16384-token batch dim, 2048 tokens/core on
8 cores, weight replicated. Per core:
  - W^T ([in=1024, out=1024], host-assembled) is DMA'd once, rounded to
    fp32r in SBUF (fp32r matmuls run at full PE rate; plain fp32 is 4x
    slower).
  - For each 128-token tile: DMA the x tile [128, 1024], PE-transpose its
    eight [128,128] sub-tiles (contraction dim must be on partitions),
    evict pairs from PSUM with DVE copies (which round to fp32r), then 16
    fp32r matmuls (free dim 512) accumulate over the 8 contraction tiles
    into PSUM [128 tok, 1024 out].
  - One DVE tensor_scalar (max 0, min 6) evicts PSUM -> SBUF with the
    ReLUN fused, then DMA out.
  - DMA traffic alternates between the two TRN2 HWDGE queues (sync,
    scalar) — a single queue's descriptor dispatch caps at ~170 GB/s.
"""

import numpy as np

N_TOKENS = 16384
D_IN = 1024
D_OUT = 1024
N_CORES = 8
TOK = N_TOKENS // N_CORES  # 2048 tokens per core
P = 128
K_TILES = D_IN // P   # 8
T_TILES = TOK // P    # 16
FD = 512              # matmul free-dim chunk (one PSUM bank of fp32)
PIPE = 4              # token tiles of transpose lookahead ahead of matmul

_CACHE = {}
LAST_RESULTS = None


def _build():
    import concourse.bacc as bacc
    import concourse.tile as tile
    import concourse.mybir as mybir
    from concourse.masks import make_identity

    f32 = mybir.dt.float32
    f32r = mybir.dt.float32r

    nc = bacc.Bacc("TRN2", target_bir_lowering=False, debug=False)
    x = nc.dram_tensor("x", [TOK, D_IN], f32, kind="ExternalInput").ap()
    wt = nc.dram_tensor("wt", [D_IN, D_OUT], f32, kind="ExternalInput").ap()
    y = nc.dram_tensor("y", [TOK, D_OUT], f32, kind="ExternalOutput").ap()

    def dma_eng(parity):
        return nc.sync if parity == 0 else nc.scalar

    with tile.TileContext(nc) as tc:
        with (
            tc.tile_pool(name="cpool", bufs=1) as cpool,
            tc.tile_pool(name="wpool", bufs=1) as wpool,
            tc.tile_pool(name="xpool", bufs=4) as xpool,
            tc.tile_pool(name="xtpool", bufs=10) as xtpool,
            tc.tile_pool(name="ypool", bufs=3) as ypool,
            tc.tile_pool(name="pstpool", bufs=4, space="PSUM") as pstpool,
            tc.tile_pool(name="psypool", bufs=2, space="PSUM") as psypool,
        ):
            ident = cpool.tile([P, P], f32)
            make_identity(nc, ident)

            # Full replicated weight in SBUF: [128 (i within tile), k-tile, out]
            w_sb = wpool.tile([P, K_TILES, D_OUT], f32r)

            xt_of = {}    # t -> list of 4 [128, 256] fp32r tiles (k-pairs)
            psy_of = {}   # t -> accumulation PSUM tile

            def emit_load_and_transpose(t):
                x_sb = xpool.tile([P, D_IN], f32, tag="x")
                dma_eng(t % 2).dma_start(x_sb[:], x[t * P:(t + 1) * P, :])
                if t == 0:
                    # Weight loads right after the first x tile on each
                    # queue; matmuls consume w_k in k order. Casts to fp32r
                    # run on GpSimd (SBUF->SBUF, 1-input is line-rate) to
                    # keep DVE free for PSUM evictions.
                    for k in range(K_TILES):
                        w_raw = xpool.tile([P, D_OUT], f32, tag="wraw", bufs=4)
                        dma_eng(k % 2).dma_start(
                            w_raw[:], wt[k * P:(k + 1) * P, :])
                        nc.vector.tensor_copy(out=w_sb[:, k], in_=w_raw[:])
                pairs = []
                for kp in range(K_TILES // 4):
                    pst = pstpool.tile([P, 4 * P], f32, tag="pst")
                    for j in range(4):
                        k = 4 * kp + j
                        nc.tensor.transpose(
                            pst[:, j * P:(j + 1) * P],
                            x_sb[:, k * P:(k + 1) * P], ident[:])
                    xt = xtpool.tile([P, 4 * P], f32r, tag="xt")
                    nc.vector.tensor_copy(out=xt[:], in_=pst[:])
                    pairs.append(xt)
                xt_of[t] = pairs

            def emit_matmuls(t):
                psy = psypool.tile([P, D_OUT], f32, tag="psy")
                for k in range(K_TILES):
                    lhsT = xt_of[t][k // 4][:, (k % 4) * P:(k % 4 + 1) * P]
                    for c in range(D_OUT // FD):
                        nc.tensor.matmul(
                            psy[:, c * FD:(c + 1) * FD],
                            lhsT,
                            w_sb[:, k, c * FD:(c + 1) * FD],
                            start=(k == 0),
                            stop=(k == K_TILES - 1),
                        )
                del xt_of[t]
                psy_of[t] = psy

            def emit_store(t):
                psy = psy_of.pop(t)
                y_sb = ypool.tile([P, D_OUT], f32, tag="y")
                nc.vector.tensor_scalar(
                    y_sb[:], psy[:], 0.0, 6.0,
                    mybir.AluOpType.max, mybir.AluOpType.min,
                )
                dma_eng((t + 1) % 2).dma_start(y[t * P:(t + 1) * P, :], y_sb[:])

            for t in range(T_TILES):
                emit_load_and_transpose(t)
                if t >= PIPE:
                    emit_matmuls(t - PIPE)
                    emit_store(t - PIPE)
            for t in range(T_TILES - PIPE, T_TILES):
                emit_matmuls(t)
                emit_store(t)

    nc.compile()
    return nc


def kernel(x, w_blocks, _trace=False):
    global LAST_RESULTS
    from concourse import bass_utils

    nc = _CACHE.get("nc")
    if nc is None:
        nc = _CACHE["nc"] = _build()

    x = np.ascontiguousarray(np.asarray(x, dtype=np.float32))
    wb = np.asarray(w_blocks, dtype=np.float32)
    # W[8a+r, 8b+s] = wb[a, b, r, s]; wt[i, o] = W[o, i]
    wt = np.ascontiguousarray(wb.transpose(1, 3, 0, 2).reshape(D_IN, D_OUT))

    in_maps = [
        {"x": x[c * TOK:(c + 1) * TOK], "wt": wt} for c in range(N_CORES)
    ]
    res = bass_utils.run_bass_kernel_spmd(
        nc, in_maps, core_ids=list(range(N_CORES)), trace=_trace,
    )
    LAST_RESULTS = res
    return np.concatenate([res.results[c]["y"] for c in range(N_CORES)], axis=0)
